# revision 17
# baseline (speedup 1.0000x reference)
"""Trainium2 Bass kernel for nn_AttentionBlock (GroupNorm + 8-head attention).

Sharding: 8 cores = 4 batches x 2 head-groups (4 heads per core).
Each core computes GroupNorm (duplicated within a batch pair), the QKV
projection for its heads, attention, and a partial output projection.
The host sums the two partials per batch and adds bias + residual.

Device layout highlights:
  - x is fed channel-major (xT [C, N]) so the C-contraction matmuls need no
    on-device transposes.
  - qq/kk tiles stack two heads on the partition axis so the two K=64 score
    matmuls run concurrently in separate PE row groups.
  - V carries an extra ones-column so the A@V matmul also produces the
    softmax denominators (row 64 of the output PSUM).
  - Group stats are reduced across the 16 channels of each group with tiny
    G / G^T matmuls (law of total variance), keeping everything in clean
    per-partition layouts.
  - Big matmuls run as float32r (1 cycle/row); A@V runs in bf16.
"""

import ml_dtypes
import numpy as np

import concourse.bass as bass
import concourse.bacc as bacc
import concourse.tile as tile
from concourse import mybir
from concourse.bass_utils import run_bass_kernel_spmd

FP32 = mybir.dt.float32
F32R = mybir.dt.float32r
BF16 = mybir.dt.bfloat16

B, HH, WW, C = 4, 32, 32, 512
N = HH * WW              # 1024 tokens
N_HEADS = 8
HD = C // N_HEADS        # 64
N_GROUPS = 32
GS = C // N_GROUPS       # 16 channels per group
GN_EPS = 1e-6
SCALE = C ** -0.5
NHC = 4                  # heads per core
P = 128
CT = C // P              # 4 channel tiles
TT = N // P              # 8 token tiles
NCORES = 8


def _mm(nc, out, lhsT, rhs, start, stop, dt=None, tile_position=None):
    if dt is not None:
        lhsT = lhsT.bitcast(dt)
        rhs = rhs.bitcast(dt)
    nc.tensor.matmul(out, lhsT, rhs, start=start, stop=stop,
                     tile_position=tile_position)


def _build_group_mats(nc, consts):
    """G [128, 8] with G[c,g] = (c//16 == g), and GT [8, 128] transposed."""
    G = consts.tile([P, 8], FP32, name="G")
    nc.gpsimd.memset(G, 1.0)
    # affine = base + part*cm + sum(pattern): keep in_ where (affine OP 0), else fill
    nc.gpsimd.affine_select(out=G, in_=G, compare_op=mybir.AluOpType.is_ge,
                            fill=0.0, base=0, pattern=[[-GS, 8]],
                            channel_multiplier=1)
    nc.gpsimd.affine_select(out=G, in_=G, compare_op=mybir.AluOpType.is_ge,
                            fill=0.0, base=GS - 1, pattern=[[GS, 8]],
                            channel_multiplier=-1)
    GT = consts.tile([8, P], FP32, name="GT")
    nc.gpsimd.memset(GT, 1.0)
    nc.gpsimd.affine_select(out=GT, in_=GT, compare_op=mybir.AluOpType.is_ge,
                            fill=0.0, base=0, pattern=[[1, P]],
                            channel_multiplier=-GS)
    nc.gpsimd.affine_select(out=GT, in_=GT, compare_op=mybir.AluOpType.is_ge,
                            fill=0.0, base=GS - 1, pattern=[[-1, P]],
                            channel_multiplier=GS)
    return G, GT


def build_program(compile=True):
    nc = bacc.Bacc()
    xT = nc.dram_tensor("xT", [C, N], BF16, kind="ExternalInput").ap()
    wqk = nc.dram_tensor("wqk", [C, 512], BF16, kind="ExternalInput").ap()
    wv = nc.dram_tensor("wv", [C, NHC * HD], BF16, kind="ExternalInput").ap()
    wo = nc.dram_tensor("wo", [NHC * HD, C], BF16, kind="ExternalInput").ap()
    gsc = nc.dram_tensor("gsc", [C], FP32, kind="ExternalInput").ap()
    gbi = nc.dram_tensor("gbi", [C], FP32, kind="ExternalInput").ap()
    y = nc.dram_tensor("y", [N, C], FP32, kind="ExternalOutput").ap()
    rd_dram = nc.dram_tensor("rd_scratch", [NHC, N], FP32).ap()

    with tile.TileContext(nc) as tc:
        with (
            tc.tile_pool(name="consts", bufs=1) as consts,
            tc.tile_pool(name="xts", bufs=1) as xts,
            tc.tile_pool(name="wpool", bufs=1) as wpool,
            tc.tile_pool(name="qk", bufs=1) as qkpool,
            tc.tile_pool(name="vp", bufs=1) as vpool,
            tc.tile_pool(name="ep", bufs=10) as epool,
            tc.tile_pool(name="small", bufs=4) as small,
            tc.tile_pool(name="res", bufs=1) as respool,
            tc.tile_pool(name="yp", bufs=3) as ypool,
            tc.tile_pool(name="ps", bufs=1, space="PSUM") as ps,
        ):
            # PSUM budget (8 banks): big 3x2 + o/sm/y 2x1 = 8
            def ps_sm(shape):
                return ps.tile(shape, FP32, name="ps_sm", tag="o", bufs=2)

            def ps_big():
                return ps.tile([P, N], FP32, name="ps_big", tag="big", bufs=3)

            def ps_o():
                return ps.tile([HD + 1, 512], FP32, name="ps_o", tag="o", bufs=2)

            def ps_y():
                return ps.tile([P, 512], FP32, name="ps_y", tag="o", bufs=2)
            # ---------------- load inputs ----------------
            xt = []
            for k in range(CT):
                t = xts.tile([P, N], BF16, name=f"xt{k}")
                nc.sync.dma_start(out=t, in_=xT[k * P:(k + 1) * P, :])
                xt.append(t)
            wqk_sb = []
            for k in range(CT):
                t = wpool.tile([P, 512], BF16, name=f"wqk{k}")
                nc.sync.dma_start(out=t, in_=wqk[k * P:(k + 1) * P, :])
                wqk_sb.append(t)
            wv_sb = []
            for k in range(CT):
                t = wpool.tile([P, NHC * HD], BF16, name=f"wv{k}")
                nc.sync.dma_start(out=t, in_=wv[k * P:(k + 1) * P, :])
                wv_sb.append(t)
            wo_sb = []
            for p in range(2):
                t = wpool.tile([P, 512], BF16, name=f"wo{p}")
                nc.sync.dma_start(out=t, in_=wo[p * P:(p + 1) * P, :])
                wo_sb.append(t)
            # gn scale/bias -> [128, 4] (column t = channels 128t..128t+127)
            gs4 = consts.tile([P, CT], FP32, name="gs4")
            gb4 = consts.tile([P, CT], FP32, name="gb4")
            nc.sync.dma_start(
                out=gs4, in_=bass.AP(tensor=gsc.tensor, offset=gsc.offset,
                                     ap=[[1, P], [P, CT]]))
            nc.sync.dma_start(
                out=gb4, in_=bass.AP(tensor=gbi.tensor, offset=gbi.offset,
                                     ap=[[1, P], [P, CT]]))

            G, GT = _build_group_mats(nc, consts)
            eps_t = consts.tile([P, 1], FP32, name="eps")
            nc.vector.memset(eps_t, GN_EPS)

            # PE warmup: dense dummy matmuls while GroupNorm runs on DVE so
            # the HAM clock gate is at 2.4 GHz when the real matmuls start.
            warm_ps = ps_o()
            for i in range(14):
                _mm(nc, warm_ps, wqk_sb[0][:, 0:HD + 1], wqk_sb[0][:, 0:512],
                    True, True)

            # ---------------- GroupNorm ----------------
            # per-channel (mean, var, mean^2) via bn_stats
            mv = []
            for k in range(CT):
                st = small.tile([P, 2, 6], FP32, name="bnst")
                nc.vector.bn_stats(out=st[:, 0, :], in_=xt[k][:, 0:512])
                nc.vector.bn_stats(out=st[:, 1, :], in_=xt[k][:, 512:1024])
                m = small.tile([P, 3], FP32, name="mv")
                nc.vector.bn_aggr(out=m[:, 0:2], in_=st)
                nc.vector.tensor_mul(m[:, 2:3], m[:, 0:1], m[:, 0:1])
                mv.append(m)
            # group sums: gps[g, 3k:3k+3] = sum over channels of tile k
            gps = ps_sm([8, 3 * CT])
            for k in range(CT):
                _mm(nc, gps[:, 3 * k:3 * k + 3], G, mv[k], True, True)
            gsb = consts.tile([8, 3 * CT], FP32, name="gsb")
            nc.vector.tensor_copy(gsb, gps)
            # expand group sums back to channels: mvx [128, (k, 3)]
            mvx_ps = ps_sm([P, 3 * CT])
            _mm(nc, mvx_ps, GT, gsb, True, True)
            mvx = consts.tile([P, CT, 3], FP32, name="mvx")
            nc.vector.tensor_copy(mvx, mvx_ps.rearrange("p (k s) -> p k s", s=3))
            # per-channel math, batched over the 4 channel tiles
            m4 = consts.tile([P, CT], FP32, name="m4")
            t4 = consts.tile([P, CT], FP32, name="t4")
            v4 = consts.tile([P, CT], FP32, name="v4")
            ab = consts.tile([P, CT, 2], FP32, name="ab")
            nc.vector.tensor_scalar_mul(m4, mvx[:, :, 0], 1.0 / GS)
            nc.vector.tensor_add(t4, mvx[:, :, 1], mvx[:, :, 2])
            nc.vector.tensor_scalar_mul(t4, t4, 1.0 / GS)
            nc.vector.tensor_mul(v4, m4, m4)
            nc.vector.tensor_sub(v4, t4, v4)          # var_g per channel
            nc.scalar.activation(out=v4, in_=v4,
                                 func=mybir.ActivationFunctionType.Sqrt,
                                 bias=eps_t, scale=1.0)
            nc.vector.reciprocal(v4, v4)              # rstd per channel
            nc.vector.tensor_mul(ab[:, :, 0], v4, gs4)           # alpha
            nc.vector.tensor_mul(t4, m4, ab[:, :, 0])
            nc.vector.tensor_sub(ab[:, :, 1], gb4, t4)           # beta
            # normalize: xn = x * alpha + beta  (bf16 for the matmuls)
            xn = []
            for k in range(CT):
                xnk = xts.tile([P, N], BF16, name=f"xn{k}")
                nc.vector.tensor_scalar(
                    out=xnk, in0=xt[k],
                    scalar1=ab[:, k, 0:1], scalar2=ab[:, k, 1:2],
                    op0=mybir.AluOpType.mult, op1=mybir.AluOpType.add)
                xn.append(xnk)

            # ---------------- QKV projection ----------------
            # qq/kk: psum [128, 1024] per m-tile; m = 0:qq0 1:kk0 2:qq1 3:kk1
            qq = [qkpool.tile([P, N], BF16, name=f"qq{p}") for p in range(2)]
            kk = [qkpool.tile([P, N], BF16, name=f"kk{p}") for p in range(2)]
            dest = [qq[0], kk[0], qq[1], kk[1]]
            for m in range(4):
                pqk = ps_big()
                for k in range(CT):
                    for h in range(2):
                        _mm(nc, pqk[:, h * 512:(h + 1) * 512],
                            wqk_sb[k][:, m * P:(m + 1) * P],
                            xn[k][:, h * 512:(h + 1) * 512],
                            k == 0, k == CT - 1)
                nc.scalar.copy(dest[m], pqk)
            # V with ones column: v1[t] [128, 4, 65] bf16
            v1 = []
            for t in range(TT):
                pvt = ps_big()
                pv = pvt[:, 0:NHC * HD]
                for k in range(CT):
                    _mm(nc, pv, xn[k][:, t * P:(t + 1) * P], wv_sb[k],
                        k == 0, k == CT - 1)
                vt = vpool.tile([P, NHC, HD + 1], BF16, name=f"v1_{t}")
                nc.scalar.copy(
                    vt[:, :, 0:HD], pv.rearrange("p (h d) -> p h d", d=HD))
                nc.vector.memset(vt[:, :, HD:HD + 1], 1.0)
                v1.append(vt)

            # ---------------- attention ----------------
            resT = [respool.tile([P, N], BF16, name=f"res{p}") for p in range(2)]
            for p in range(2):
                for q in range(2):  # head within pair
                    lh = 2 * p + q          # local head index
                    o_h = [ps_o(), ps_o()]  # one [65, 512] accumulator per i-half
                    for jt in range(TT):
                        s_ps = ps_big()
                        for ih in range(2):
                            _mm(nc, s_ps[:, ih * 512:(ih + 1) * 512],
                                kk[p][q * HD:(q + 1) * HD, jt * P:(jt + 1) * P],
                                qq[p][q * HD:(q + 1) * HD,
                                      ih * 512:(ih + 1) * 512],
                                True, True,
                                tile_position=(q * HD, 0))
                        e_t = epool.tile([P, N], BF16, name="e")
                        nc.scalar.activation(out=e_t, in_=s_ps,
                                             func=mybir.ActivationFunctionType.Exp,
                                             scale=SCALE)
                        for ih in range(2):
                            _mm(nc, o_h[ih],
                                v1[jt][:, lh, :],
                                e_t[:, ih * 512:(ih + 1) * 512],
                                jt == 0, jt == TT - 1)
                    # stage O in SBUF so the PSUM accumulators free early,
                    # then divide rows 0..63 by the denominator row (row 64):
                    # repartition D to [128, 8] so the reciprocal runs wide,
                    # then broadcast 1/D across 64 partitions via DRAM.
                    o_sb = small.tile([HD + 1, N], FP32, name="o_sb")
                    for ih in range(2):
                        nc.vector.tensor_copy(
                            o_sb[:, ih * 512:(ih + 1) * 512], o_h[ih])
                    rdp = small.tile([P, TT], FP32, name="rdp")
                    nc.sync.dma_start(out=rdp, in_=o_sb[HD:HD + 1, :])
                    nc.vector.reciprocal(rdp, rdp)
                    nc.sync.dma_start(out=rd_dram[lh:lh + 1, :], in_=rdp)
                    rdb = small.tile([HD, N], FP32, name="rdb")
                    nc.sync.dma_start(
                        out=rdb,
                        in_=bass.AP(tensor=rd_dram.tensor, offset=lh * N,
                                    ap=[[0, HD], [1, N]]))
                    nc.vector.tensor_mul(resT[p][q * HD:(q + 1) * HD, :],
                                         o_sb[0:HD, :], rdb)

            # ---------------- output projection ----------------
            for it in range(TT):
                yp = ps_y()
                for p in range(2):
                    _mm(nc, yp, resT[p][:, it * P:(it + 1) * P], wo_sb[p],
                        p == 0, p == 1)
                ysb = ypool.tile([P, 512], FP32, name="ysb")
                nc.vector.tensor_copy(ysb, yp)
                nc.sync.dma_start(out=y[it * P:(it + 1) * P, :], in_=ysb)
    if compile:
        nc.compile()
        nc.finalize()
    return nc


_CACHE = {}


def _get_program():
    if "nc" not in _CACHE:
        _CACHE["nc"] = build_program()
    return _CACHE["nc"]


def make_in_maps(x, gn_scale, gn_bias, w_qkv, w_out):
    x = np.ascontiguousarray(x, dtype=np.float32)
    w_qkv = np.asarray(w_qkv, dtype=np.float32)
    w_out = np.asarray(w_out, dtype=np.float32)
    gn_scale = np.asarray(gn_scale, dtype=np.float32)
    gn_bias = np.asarray(gn_bias, dtype=np.float32)
    # per-head column blocks of w_qkv: head h -> [q | k | v] at 3*HD*h
    qcols = [w_qkv[:, 3 * HD * h:3 * HD * h + HD] for h in range(N_HEADS)]
    kcols = [w_qkv[:, 3 * HD * h + HD:3 * HD * h + 2 * HD] for h in range(N_HEADS)]
    vcols = [w_qkv[:, 3 * HD * h + 2 * HD:3 * HD * h + 3 * HD] for h in range(N_HEADS)]
    in_maps = []
    for cid in range(NCORES):
        b, hg = divmod(cid, 2)
        hs = [4 * hg + l for l in range(NHC)]
        xb = x[b].reshape(N, C)
        wqk = np.concatenate(
            [qcols[hs[0]], qcols[hs[1]], kcols[hs[0]], kcols[hs[1]],
             qcols[hs[2]], qcols[hs[3]], kcols[hs[2]], kcols[hs[3]]], axis=1)
        wv = np.concatenate([vcols[h] for h in hs], axis=1)
        wo = np.concatenate([w_out[HD * h:HD * (h + 1), :] for h in hs], axis=0)
        in_maps.append({
            "xT": np.ascontiguousarray(xb.T.astype(ml_dtypes.bfloat16)),
            "wqk": np.ascontiguousarray(wqk.astype(ml_dtypes.bfloat16)),
            "wv": np.ascontiguousarray(wv.astype(ml_dtypes.bfloat16)),
            "wo": np.ascontiguousarray(wo.astype(ml_dtypes.bfloat16)),
            "gsc": gn_scale,
            "gbi": gn_bias,
        })
    return in_maps


def kernel(x, gn_scale, gn_bias, w_qkv, w_out, b_out, _trace=False, _trace_kwargs=None):
    x = np.asarray(x, dtype=np.float32)
    b_out = np.asarray(b_out, dtype=np.float32)
    nc = _get_program()
    in_maps = make_in_maps(x, gn_scale, gn_bias, w_qkv, w_out)
    kw = {}
    if _trace:
        kw = dict(trace=True, **(_trace_kwargs or {}))
    res = run_bass_kernel_spmd(nc, in_maps, list(range(NCORES)), **kw)
    _CACHE["last_results"] = res
    out = np.empty((B, N, C), np.float32)
    for b in range(B):
        out[b] = res.results[2 * b]["y"] + res.results[2 * b + 1]["y"]
        out[b] += x[b].reshape(N, C) + b_out
    return out.reshape(B, HH, WW, C)


# revision 20
# speedup vs baseline: 1.0973x; 1.0973x over previous
"""Trainium2 Bass kernel for nn_AttentionBlock (GroupNorm + 8-head attention).

Sharding: 8 cores = 4 batches x 2 head-groups (4 heads per core).
Each core computes GroupNorm (duplicated within a batch pair), the QKV
projection for its heads, attention, and a partial output projection.
The host sums the two partials per batch and adds bias + residual.

Device layout highlights:
  - x is fed channel-major (xT [C, N]) so the C-contraction matmuls need no
    on-device transposes.
  - qq/kk tiles stack two heads on the partition axis so the two K=64 score
    matmuls run concurrently in separate PE row groups.
  - V carries an extra ones-column so the A@V matmul also produces the
    softmax denominators (row 64 of the output PSUM).
  - Group stats are reduced across the 16 channels of each group with tiny
    G / G^T matmuls (law of total variance), keeping everything in clean
    per-partition layouts.
  - Big matmuls run as float32r (1 cycle/row); A@V runs in bf16.
"""

import ml_dtypes
import numpy as np

import concourse.bass as bass
import concourse.bacc as bacc
import concourse.tile as tile
from concourse import mybir
from concourse.bass_utils import run_bass_kernel_spmd

FP32 = mybir.dt.float32
F32R = mybir.dt.float32r
BF16 = mybir.dt.bfloat16

B, HH, WW, C = 4, 32, 32, 512
N = HH * WW              # 1024 tokens
N_HEADS = 8
HD = C // N_HEADS        # 64
N_GROUPS = 32
GS = C // N_GROUPS       # 16 channels per group
GN_EPS = 1e-6
SCALE = C ** -0.5
NHC = 4                  # heads per core
P = 128
CT = C // P              # 4 channel tiles
TT = N // P              # 8 token tiles
NCORES = 8


def _mm(nc, out, lhsT, rhs, start, stop, dt=None, tile_position=None):
    if dt is not None:
        lhsT = lhsT.bitcast(dt)
        rhs = rhs.bitcast(dt)
    nc.tensor.matmul(out, lhsT, rhs, start=start, stop=stop,
                     tile_position=tile_position)


def _build_group_mats(nc, consts):
    """G [128, 8] with G[c,g] = (c//16 == g), and GT [8, 128] transposed."""
    G = consts.tile([P, 8], FP32, name="G")
    nc.gpsimd.memset(G, 1.0)
    # affine = base + part*cm + sum(pattern): keep in_ where (affine OP 0), else fill
    nc.gpsimd.affine_select(out=G, in_=G, compare_op=mybir.AluOpType.is_ge,
                            fill=0.0, base=0, pattern=[[-GS, 8]],
                            channel_multiplier=1)
    nc.gpsimd.affine_select(out=G, in_=G, compare_op=mybir.AluOpType.is_ge,
                            fill=0.0, base=GS - 1, pattern=[[GS, 8]],
                            channel_multiplier=-1)
    GT = consts.tile([8, P], FP32, name="GT")
    nc.gpsimd.memset(GT, 1.0)
    nc.gpsimd.affine_select(out=GT, in_=GT, compare_op=mybir.AluOpType.is_ge,
                            fill=0.0, base=0, pattern=[[1, P]],
                            channel_multiplier=-GS)
    nc.gpsimd.affine_select(out=GT, in_=GT, compare_op=mybir.AluOpType.is_ge,
                            fill=0.0, base=GS - 1, pattern=[[-1, P]],
                            channel_multiplier=GS)
    return G, GT


def build_program(compile=True):
    nc = bacc.Bacc()
    xT = nc.dram_tensor("xT", [C, N], BF16, kind="ExternalInput").ap()
    wqk = nc.dram_tensor("wqk", [C, 512], BF16, kind="ExternalInput").ap()
    wv = nc.dram_tensor("wv", [C, NHC * HD], BF16, kind="ExternalInput").ap()
    wo = nc.dram_tensor("wo", [NHC * HD, C], BF16, kind="ExternalInput").ap()
    gsc = nc.dram_tensor("gsc", [C], FP32, kind="ExternalInput").ap()
    gbi = nc.dram_tensor("gbi", [C], FP32, kind="ExternalInput").ap()
    y = nc.dram_tensor("y", [N, C], FP32, kind="ExternalOutput").ap()
    rd_dram = nc.dram_tensor("rd_scratch", [NHC, N], FP32).ap()

    with tile.TileContext(nc) as tc:
        with (
            tc.tile_pool(name="consts", bufs=1) as consts,
            tc.tile_pool(name="xts", bufs=1) as xts,
            tc.tile_pool(name="wpool", bufs=1) as wpool,
            tc.tile_pool(name="qk", bufs=1) as qkpool,
            tc.tile_pool(name="vp", bufs=1) as vpool,
            tc.tile_pool(name="ep", bufs=14) as epool,
            tc.tile_pool(name="small", bufs=4) as small,
            tc.tile_pool(name="res", bufs=1) as respool,
            tc.tile_pool(name="yp", bufs=3) as ypool,
            tc.tile_pool(name="ps", bufs=1, space="PSUM") as ps,
        ):
            # PSUM budget (8 banks): big 3x2 + o/sm/y 2x1 = 8
            def ps_sm(shape):
                return ps.tile(shape, FP32, name="ps_sm", tag="o", bufs=2)

            def ps_big():
                return ps.tile([P, N], FP32, name="ps_big", tag="big", bufs=3)

            def ps_o():
                return ps.tile([HD + 1, 512], FP32, name="ps_o", tag="o", bufs=2)

            def ps_y():
                return ps.tile([P, 512], FP32, name="ps_y", tag="o", bufs=2)
            # ---------------- load inputs ----------------
            xt = []
            for k in range(CT):
                t = xts.tile([P, N], BF16, name=f"xt{k}")
                nc.sync.dma_start(out=t, in_=xT[k * P:(k + 1) * P, :])
                xt.append(t)
            wqk_sb = []
            for k in range(CT):
                t = wpool.tile([P, 512], BF16, name=f"wqk{k}")
                nc.sync.dma_start(out=t, in_=wqk[k * P:(k + 1) * P, :])
                wqk_sb.append(t)
            wv_sb = []
            for k in range(CT):
                t = wpool.tile([P, NHC * HD], BF16, name=f"wv{k}")
                nc.sync.dma_start(out=t, in_=wv[k * P:(k + 1) * P, :])
                wv_sb.append(t)
            wo_sb = []
            for p in range(2):
                t = wpool.tile([P, 512], BF16, name=f"wo{p}")
                nc.sync.dma_start(out=t, in_=wo[p * P:(p + 1) * P, :])
                wo_sb.append(t)
            # gn scale/bias -> [128, 4] (column t = channels 128t..128t+127)
            gs4 = consts.tile([P, CT], FP32, name="gs4")
            gb4 = consts.tile([P, CT], FP32, name="gb4")
            nc.sync.dma_start(
                out=gs4, in_=bass.AP(tensor=gsc.tensor, offset=gsc.offset,
                                     ap=[[1, P], [P, CT]]))
            nc.sync.dma_start(
                out=gb4, in_=bass.AP(tensor=gbi.tensor, offset=gbi.offset,
                                     ap=[[1, P], [P, CT]]))

            G, GT = _build_group_mats(nc, consts)
            eps_t = consts.tile([P, 1], FP32, name="eps")
            nc.vector.memset(eps_t, GN_EPS)

            # PE warmup: dense dummy matmuls while GroupNorm runs on DVE so
            # the HAM clock gate is at 2.4 GHz when the real matmuls start.
            warm_ps = ps_o()
            for i in range(14):
                _mm(nc, warm_ps, wqk_sb[0][:, 0:HD + 1], wqk_sb[0][:, 0:512],
                    True, True)

            # ---------------- GroupNorm ----------------
            # per-channel (mean, var, mean^2) via bn_stats
            mv = []
            for k in range(CT):
                st = small.tile([P, 2, 6], FP32, name="bnst")
                nc.vector.bn_stats(out=st[:, 0, :], in_=xt[k][:, 0:512])
                nc.vector.bn_stats(out=st[:, 1, :], in_=xt[k][:, 512:1024])
                m = small.tile([P, 3], FP32, name="mv")
                nc.vector.bn_aggr(out=m[:, 0:2], in_=st)
                nc.vector.tensor_mul(m[:, 2:3], m[:, 0:1], m[:, 0:1])
                mv.append(m)
            # group sums: gps[g, 3k:3k+3] = sum over channels of tile k
            gps = ps_sm([8, 3 * CT])
            for k in range(CT):
                _mm(nc, gps[:, 3 * k:3 * k + 3], G, mv[k], True, True)
            gsb = consts.tile([8, 3 * CT], FP32, name="gsb")
            nc.vector.tensor_copy(gsb, gps)
            # expand group sums back to channels: mvx [128, (k, 3)]
            mvx_ps = ps_sm([P, 3 * CT])
            _mm(nc, mvx_ps, GT, gsb, True, True)
            mvx = consts.tile([P, CT, 3], FP32, name="mvx")
            nc.vector.tensor_copy(mvx, mvx_ps.rearrange("p (k s) -> p k s", s=3))
            # per-channel math, batched over the 4 channel tiles
            m4 = consts.tile([P, CT], FP32, name="m4")
            t4 = consts.tile([P, CT], FP32, name="t4")
            v4 = consts.tile([P, CT], FP32, name="v4")
            ab = consts.tile([P, CT, 2], FP32, name="ab")
            nc.vector.tensor_scalar_mul(m4, mvx[:, :, 0], 1.0 / GS)
            nc.vector.tensor_add(t4, mvx[:, :, 1], mvx[:, :, 2])
            nc.vector.tensor_scalar_mul(t4, t4, 1.0 / GS)
            nc.vector.tensor_mul(v4, m4, m4)
            nc.vector.tensor_sub(v4, t4, v4)          # var_g per channel
            nc.scalar.activation(out=v4, in_=v4,
                                 func=mybir.ActivationFunctionType.Sqrt,
                                 bias=eps_t, scale=1.0)
            nc.vector.reciprocal(v4, v4)              # rstd per channel
            nc.vector.tensor_mul(ab[:, :, 0], v4, gs4)           # alpha
            nc.vector.tensor_mul(t4, m4, ab[:, :, 0])
            nc.vector.tensor_sub(ab[:, :, 1], gb4, t4)           # beta
            # normalize: xn = x * alpha + beta  (bf16 for the matmuls)
            xn = []
            for k in range(CT):
                xnk = xts.tile([P, N], BF16, name=f"xn{k}")
                nc.vector.tensor_scalar(
                    out=xnk, in0=xt[k],
                    scalar1=ab[:, k, 0:1], scalar2=ab[:, k, 1:2],
                    op0=mybir.AluOpType.mult, op1=mybir.AluOpType.add)
                xn.append(xnk)

            # -------- QKV + attention, software-pipelined across heads ------
            # step s: scores+exp for head s, A@V for head s-1. The V
            # projection matmuls interleave with head-0 scores and the
            # pair-1 qq/kk matmuls interleave with head-1 scores, so the PE
            # sees a dense instruction stream and stays at 2.4 GHz.
            qq = [qkpool.tile([P, N], BF16, name=f"qq{p}") for p in range(2)]
            kk = [qkpool.tile([P, N], BF16, name=f"kk{p}") for p in range(2)]
            dest = [qq[0], kk[0], qq[1], kk[1]]
            resT = [respool.tile([P, N], BF16, name=f"res{p}") for p in range(2)]
            ones_r = consts.tile([1, HD], BF16, name="ones_r")
            nc.vector.memset(ones_r, 1.0)
            v1 = [None] * TT
            e_hold = [[None] * TT for _ in range(NHC)]
            o_hold = [None] * NHC

            def emit_qk(m):
                pqk = ps_big()
                for k in range(CT):
                    for h in range(2):
                        _mm(nc, pqk[:, h * 512:(h + 1) * 512],
                            wqk_sb[k][:, m * P:(m + 1) * P],
                            xn[k][:, h * 512:(h + 1) * 512],
                            k == 0, k == CT - 1)
                nc.vector.tensor_copy(dest[m], pqk)

            def emit_v(t):
                pvt = ps_big()
                pv = pvt[:, 0:NHC * HD]
                for k in range(CT):
                    _mm(nc, pv, xn[k][:, t * P:(t + 1) * P], wv_sb[k],
                        k == 0, k == CT - 1)
                vt = vpool.tile([P, NHC, HD + 1], BF16, name=f"v1_{t}")
                nc.vector.tensor_copy(
                    vt[:, :, 0:HD], pv.rearrange("p (h d) -> p h d", d=HD))
                nc.vector.memset(vt[:, :, HD:HD + 1], 1.0)
                v1[t] = vt

            def emit_scores(h, jt):
                p, q = divmod(h, 2)
                s_ps = ps_big()
                for ih in range(2):
                    _mm(nc, s_ps[:, ih * 512:(ih + 1) * 512],
                        kk[p][q * HD:(q + 1) * HD, jt * P:(jt + 1) * P],
                        qq[p][q * HD:(q + 1) * HD, ih * 512:(ih + 1) * 512],
                        True, True, tile_position=(q * HD, 0))
                e_t = epool.tile([P, N], BF16, name="e")
                nc.scalar.activation(out=e_t, in_=s_ps,
                                     func=mybir.ActivationFunctionType.Exp,
                                     scale=SCALE)
                e_hold[h][jt] = e_t

            def emit_av(h, jt):
                if jt == 0:
                    o_hold[h] = [ps_o(), ps_o()]
                for ih in range(2):
                    _mm(nc, o_hold[h][ih], v1[jt][:, h, :],
                        e_hold[h][jt][:, ih * 512:(ih + 1) * 512],
                        jt == 0, jt == TT - 1)
                e_hold[h][jt] = None

            def drain(h):
                # resT rows = O[0:64] * (1/D): repartition the D row to
                # [128, 8] so the reciprocal runs wide, then broadcast 1/D
                # across 64 partitions via DRAM.
                p, q = divmod(h, 2)
                o_pair = o_hold[h]
                o_sb = small.tile([HD + 1, N], FP32, name="o_sb")
                for ih in range(2):
                    nc.vector.tensor_copy(
                        o_sb[:, ih * 512:(ih + 1) * 512], o_pair[ih])
                rdp = small.tile([P, TT], FP32, name="rdp")
                nc.sync.dma_start(out=rdp, in_=o_sb[HD:HD + 1, :])
                nc.vector.reciprocal(rdp, rdp)
                nc.sync.dma_start(out=rd_dram[h:h + 1, :], in_=rdp)
                rdb = small.tile([HD, N], FP32, name="rdb")
                nc.sync.dma_start(
                    out=rdb,
                    in_=bass.AP(tensor=rd_dram.tensor, offset=h * N,
                                ap=[[0, HD], [1, N]]))
                nc.vector.tensor_mul(resT[p][q * HD:(q + 1) * HD, :],
                                     o_sb[0:HD, :], rdb)

            emit_qk(0)
            emit_qk(1)
            for step in range(NHC + 1):
                for jt in range(TT):
                    if step < NHC:
                        emit_scores(step, jt)
                    if step == 0:
                        emit_v(jt)
                    if step == 1 and jt < 2:
                        emit_qk(2 + jt)
                    if step >= 1:
                        emit_av(step - 1, jt)
                if step >= 1:
                    drain(step - 1)

            # ---------------- output projection ----------------
            for it in range(TT):
                yp = ps_y()
                for p in range(2):
                    _mm(nc, yp, resT[p][:, it * P:(it + 1) * P], wo_sb[p],
                        p == 0, p == 1)
                ysb = ypool.tile([P, 512], FP32, name="ysb")
                nc.vector.tensor_copy(ysb, yp)
                nc.sync.dma_start(out=y[it * P:(it + 1) * P, :], in_=ysb)
    if compile:
        nc.compile()
        nc.finalize()
    return nc


_CACHE = {}


def _get_program():
    if "nc" not in _CACHE:
        _CACHE["nc"] = build_program()
    return _CACHE["nc"]


def make_in_maps(x, gn_scale, gn_bias, w_qkv, w_out):
    x = np.ascontiguousarray(x, dtype=np.float32)
    w_qkv = np.asarray(w_qkv, dtype=np.float32)
    w_out = np.asarray(w_out, dtype=np.float32)
    gn_scale = np.asarray(gn_scale, dtype=np.float32)
    gn_bias = np.asarray(gn_bias, dtype=np.float32)
    # per-head column blocks of w_qkv: head h -> [q | k | v] at 3*HD*h
    qcols = [w_qkv[:, 3 * HD * h:3 * HD * h + HD] for h in range(N_HEADS)]
    kcols = [w_qkv[:, 3 * HD * h + HD:3 * HD * h + 2 * HD] for h in range(N_HEADS)]
    vcols = [w_qkv[:, 3 * HD * h + 2 * HD:3 * HD * h + 3 * HD] for h in range(N_HEADS)]
    in_maps = []
    for cid in range(NCORES):
        b, hg = divmod(cid, 2)
        hs = [4 * hg + l for l in range(NHC)]
        xb = x[b].reshape(N, C)
        wqk = np.concatenate(
            [qcols[hs[0]], qcols[hs[1]], kcols[hs[0]], kcols[hs[1]],
             qcols[hs[2]], qcols[hs[3]], kcols[hs[2]], kcols[hs[3]]], axis=1)
        wv = np.concatenate([vcols[h] for h in hs], axis=1)
        wo = np.concatenate([w_out[HD * h:HD * (h + 1), :] for h in hs], axis=0)
        in_maps.append({
            "xT": np.ascontiguousarray(xb.T.astype(ml_dtypes.bfloat16)),
            "wqk": np.ascontiguousarray(wqk.astype(ml_dtypes.bfloat16)),
            "wv": np.ascontiguousarray(wv.astype(ml_dtypes.bfloat16)),
            "wo": np.ascontiguousarray(wo.astype(ml_dtypes.bfloat16)),
            "gsc": gn_scale,
            "gbi": gn_bias,
        })
    return in_maps


def kernel(x, gn_scale, gn_bias, w_qkv, w_out, b_out, _trace=False, _trace_kwargs=None):
    x = np.asarray(x, dtype=np.float32)
    b_out = np.asarray(b_out, dtype=np.float32)
    nc = _get_program()
    in_maps = make_in_maps(x, gn_scale, gn_bias, w_qkv, w_out)
    kw = {}
    if _trace:
        kw = dict(trace=True, **(_trace_kwargs or {}))
    res = run_bass_kernel_spmd(nc, in_maps, list(range(NCORES)), **kw)
    _CACHE["last_results"] = res
    out = np.empty((B, N, C), np.float32)
    for b in range(B):
        out[b] = res.results[2 * b]["y"] + res.results[2 * b + 1]["y"]
        out[b] += x[b].reshape(N, C) + b_out
    return out.reshape(B, HH, WW, C)
